# revision 21
# baseline (speedup 1.0000x reference)
"""AffineCoupling (NICE) forward on 8 Trainium2 NeuronCores.

Data-parallel over the batch: each core gets a 1024-row slice of x and
runs the full 6-layer MLP (512->2048->2048x4->1024) with replicated
weights, then the coupling epilogue (tanh / exp / scale+shift /
log-det) on chip.

Layout: activations are kept feature-major (hT[feat, batch]) so the
weight matrices, stored [in, out], are directly the stationary matmul
operand and the chain needs no per-layer transposes.  Matmuls run in
fp32r (FP22-precision fp32, full PE rate at N=512).  The even/odd
feature split of the coupling is folded into the M-tiling of the last
layer, so tanh/exp/shift operate on contiguous tiles.
"""

import sys

for _p in ("/opt/trn_rl_repo",):
    if _p not in sys.path:
        sys.path.insert(0, _p)

from contextlib import ExitStack

import numpy as np

import concourse.bass as bass
import concourse.tile as tile
from concourse import bacc, mybir
from concourse.bass_utils import run_bass_kernel_spmd
from concourse.masks import make_identity

P = 128
NCORES = 8
B = 8192  # full batch
W = 1024  # feature width
BC = B // NCORES  # batch rows per core (1024)
NT = 512  # matmul moving-dim tile (max for fp32, = 1 PSUM bank)
MID = 2048
F32 = mybir.dt.float32
F32R = mybir.dt.float32r
AF = mybir.ActivationFunctionType

# (in_dim, out_dim) per layer
L_DIMS = [(W // 2, MID)] + [(MID, MID)] * 4 + [(MID, W)]


def _build_nc():
    nc = bacc.Bacc(trn_type="TRN2", target_bir_lowering=False, debug=False)

    x_d = nc.dram_tensor("x", [BC, W], F32, kind="ExternalInput").ap()
    ldj_d = nc.dram_tensor("log_det_J", [BC], F32, kind="ExternalInput").ap()
    w_d = []
    b_d = []
    for i, (di, do) in enumerate(L_DIMS):
        w_d.append(nc.dram_tensor(f"w{i}", [di, do], F32, kind="ExternalInput").ap())
        b_d.append(nc.dram_tensor(f"b{i}", [do], F32, kind="ExternalInput").ap())
    y_d = nc.dram_tensor("y", [BC, W], F32, kind="ExternalOutput").ap()
    ld_d = nc.dram_tensor("log_det", [BC], F32, kind="ExternalOutput").ap()

    with tile.TileContext(nc) as tc, ExitStack() as ctx:
        const = ctx.enter_context(tc.tile_pool(name="const", bufs=1))
        xt_pool = ctx.enter_context(tc.tile_pool(name="xt", bufs=1))
        xin_pool = ctx.enter_context(tc.tile_pool(name="xin", bufs=2))
        h_pool = ctx.enter_context(tc.tile_pool(name="h", bufs=2))
        w_pool = ctx.enter_context(tc.tile_pool(name="w", bufs=4))
        mm_ps = ctx.enter_context(tc.tile_pool(name="mmps", bufs=4, space="PSUM"))
        tr_ps = ctx.enter_context(tc.tile_pool(name="trps", bufs=2, space="PSUM"))
        ld_ps = ctx.enter_context(tc.tile_pool(name="ldps", bufs=2, space="PSUM"))

        ident_f = const.tile([P, P], F32, tag="identf")
        make_identity(nc, ident_f)
        ident = const.tile([P, P], F32R, tag="ident")
        nc.vector.tensor_copy(ident[:], ident_f[:])
        ones_f = const.tile([P, 1], F32, tag="onesf")
        nc.gpsimd.memset(ones_f[:], 1.0)
        ones = const.tile([P, 1], F32R, tag="ones")
        nc.vector.tensor_copy(ones[:], ones_f[:])

        # Biases, feature-major: column m holds features m*128..m*128+127.
        bias_sb = []
        for l, (_, do) in enumerate(L_DIMS[:5]):
            bt = const.tile([P, do // P], F32, tag=f"bias{l}")
            nc.gpsimd.dma_start(bt[:], b_d[l].rearrange("(mo p) -> p mo", p=P))
            bias_sb.append(bt)
        b5r = b_d[5].rearrange("(mo p two) -> p mo two", p=P, two=2)
        b5e = const.tile([P, 4], F32, tag="b5e")
        nc.gpsimd.dma_start(b5e[:], b5r[:, :, 0])
        b5o = const.tile([P, 4], F32, tag="b5o")
        nc.gpsimd.dma_start(b5o[:], b5r[:, :, 1])
        ld_sb = const.tile([1, BC], F32, tag="ldout")
        nc.gpsimd.dma_start(ld_sb[:], ldj_d.unsqueeze(0))

        # Weight loads: half-pairs [P, <=8, 256] so the first 8 k-chunks are
        # usable while the rest stream; 4 bufs = two m-pairs in flight.
        w_rearr = [w.rearrange("(ko p) m -> p ko m", p=P) for w in w_d]

        def load_pair(l_idx, mp):
            ko = L_DIMS[l_idx][0] // P
            wr = w_rearr[l_idx]
            halves = []
            for h0 in range(0, ko, 8):
                hk = min(8, ko - h0)
                wt = w_pool.tile(
                    [P, 8, 256], F32R, tag="w", name=f"w{l_idx}_{mp}_{h0}"
                )
                nc.sync.dma_start(
                    wt[:, :hk, :],
                    wr[:, h0 : h0 + hk, mp * 256 : (mp + 1) * 256].bitcast(F32R),
                )
                halves.append(wt)
            return halves

        def pair_slice(halves, k, csl):
            return halves[k // 8][:, k % 8, csl]

        def load_l0_quad(mq):
            # L0 (ko=4) packs four m-tiles (512 cols) into one w slot
            wt = w_pool.tile([P, 4, 512], F32R, tag="w", name=f"w0q{mq}")
            nc.sync.dma_start(
                wt[:],
                w_rearr[0][:, 0:4, mq * 512 : (mq + 1) * 512].bitcast(F32R),
            )
            return wt

        l0_quads = {0: load_l0_quad(0)}

        # ---- Stage A: split-transpose x into x1T (even cols) / x2T (odd) ----
        x1T = xt_pool.tile([P, 4, BC], F32R)  # [feat, chunk, batch]
        x2T = xt_pool.tile([P, 4, BC], F32)
        for bb in range(BC // P):
            for hf in range(2):
                xc = xin_pool.tile([P, W // 2], F32R, tag="xblk")
                nc.sync.dma_start(
                    xc[:],
                    x_d[
                        bb * P : (bb + 1) * P, hf * 512 : (hf + 1) * 512
                    ].bitcast(F32R),
                )
                for sh in range(2):  # 256-col spans within this half
                    s = hf * 2 + sh
                    pe = tr_ps.tile([P, P], F32R, tag="tr")
                    nc.tensor.transpose(
                        pe[:], xc[:, sh * 256 : (sh + 1) * 256 : 2], ident[:]
                    )
                    nc.vector.tensor_copy(x1T[:, s, bb * P : (bb + 1) * P], pe[:])
                    po = tr_ps.tile([P, P], F32R, tag="tr")
                    nc.tensor.transpose(
                        po[:], xc[:, sh * 256 + 1 : (sh + 1) * 256 : 2], ident[:]
                    )
                    nc.vector.tensor_copy(x2T[:, s, bb * P : (bb + 1) * P], po[:])
            # trickle the rest of L0's weight loads between x blocks so they
            # don't all queue behind the x DMAs on the Sync engine
            if bb in (1, 3, 5):
                mq = (bb + 1) // 2
                l0_quads[mq] = load_l0_quad(mq)

        cur = x1T
        h5 = None
        for l in range(6):
            di, do = L_DIMS[l]
            ko = di // P
            if l == 0:
                # All weights preloaded as 4 quads; n-outer so the n=0 groups
                # run while the second half of x (bb4-7) is still streaming.
                nxt = h_pool.tile([P, 16, BC], F32R, tag="h")
                for n in range(BC // NT):
                    for mq in range(4):
                        wt = l0_quads[mq]
                        for j in range(4):
                            m = mq * 4 + j
                            acc = mm_ps.tile([P, NT], F32, tag="acc")
                            for k in range(ko):
                                nc.tensor.matmul(
                                    acc[:],
                                    wt[:, k, j * P : (j + 1) * P],
                                    cur[:, k, n * NT : (n + 1) * NT],
                                    start=(k == 0),
                                    stop=(k == ko - 1),
                                )
                            nc.scalar.activation(
                                nxt[:, m, n * NT : (n + 1) * NT],
                                acc[:],
                                AF.Relu,
                                bias=bias_sb[0][:, m : m + 1],
                            )
                cur = nxt
            elif l < 5:
                nxt = h_pool.tile([P, 16, BC], F32R, tag="h")
                for mp in range(do // 256):  # paired m-tiles: 1KB DMA runs
                    wt = load_pair(l, mp)
                    for ms in range(2):
                        m = mp * 2 + ms
                        for n in range(BC // NT):
                            acc = mm_ps.tile([P, NT], F32, tag="acc")
                            for k in range(ko):
                                nc.tensor.matmul(
                                    acc[:],
                                    pair_slice(wt, k, slice(ms * P, (ms + 1) * P)),
                                    cur[:, k, n * NT : (n + 1) * NT],
                                    start=(k == 0),
                                    stop=(k == ko - 1),
                                )
                            nc.scalar.activation(
                                nxt[:, m, n * NT : (n + 1) * NT],
                                acc[:],
                                AF.Relu,
                                bias=bias_sb[l][:, m : m + 1],
                            )
                cur = nxt
            else:
                # Last layer: even out-features -> tanh (log-scale),
                # odd out-features -> identity+bias (shift).
                h5 = h_pool.tile([P, 16, BC], F32R, tag="h")
                b1tT = h5[:, 0:4]  # tanh output, even features
                b2tT = h5[:, 4:8]  # shift, odd features
                laccs = [
                    ld_ps.tile([1, NT], F32, tag="ld", name=f"lacc{n}")
                    for n in range(BC // NT)
                ]
                for m in range(4):  # 256-wide contiguous spans of w5 cols
                    wt = load_pair(l, m)
                    for n in range(BC // NT):
                        nsl = slice(n * NT, (n + 1) * NT)
                        acc_e = mm_ps.tile([P, NT], F32, tag="acc")
                        for k in range(ko):
                            nc.tensor.matmul(
                                acc_e[:],
                                pair_slice(wt, k, slice(0, 256, 2)),
                                cur[:, k, nsl],
                                start=(k == 0),
                                stop=(k == ko - 1),
                            )
                        nc.scalar.activation(
                            b1tT[:, m, nsl],
                            acc_e[:],
                            AF.Tanh,
                            bias=b5e[:, m : m + 1],
                        )
                        acc_o = mm_ps.tile([P, NT], F32, tag="acc")
                        for k in range(ko):
                            nc.tensor.matmul(
                                acc_o[:],
                                pair_slice(wt, k, slice(1, 256, 2)),
                                cur[:, k, nsl],
                                start=(k == 0),
                                stop=(k == ko - 1),
                            )
                        # log_det partial: sum_partitions(tanh chunk) via ones
                        # (after acc_o so the tanh eviction overlaps PE work)
                        nc.tensor.matmul(
                            laccs[n][:],
                            ones[:],
                            b1tT[:, m, nsl],
                            start=(m == 0),
                            stop=(m == 3),
                        )
                        nc.scalar.activation(
                            b2tT[:, m, nsl],
                            acc_o[:],
                            AF.Identity,
                            bias=b5o[:, m : m + 1],
                        )

        assert h5 is not None
        b1tT = h5[:, 0:4]
        b2tT = h5[:, 4:8]
        expb = h5[:, 8:12]
        y2T = h5[:, 12:16]

        # ---- log_det = log_det_J + accumulated tanh partials ----
        for n in range(BC // NT):
            nsl = slice(n * NT, (n + 1) * NT)
            nc.vector.tensor_add(ld_sb[:, nsl], laccs[n][:], ld_sb[:, nsl])
        nc.sync.dma_start(ld_d.unsqueeze(0), ld_sb[:])

        # ---- y2 = x2 * exp(b1t) + b2t (feature-major) ----
        for c in range(4):
            nc.scalar.activation(expb[:, c], b1tT[:, c], AF.Exp)
            nc.vector.tensor_mul(y2T[:, c], expb[:, c], x2T[:, c])
            nc.vector.tensor_add(y2T[:, c], y2T[:, c], b2tT[:, c])

        # ---- Stage D: transpose back + interleave into y ----
        y_combo = h_pool.tile([P, 16, BC], F32, tag="h")
        y_sb = y_combo[:, 0:8]  # [P, batch_block, 1024] batch-major
        for bb in range(BC // P):
            bsl = slice(bb * P, (bb + 1) * P)
            for s in range(4):
                pt = tr_ps.tile([P, P], F32R, tag="tr")
                nc.tensor.transpose(pt[:], y2T[:, s, bsl], ident[:])
                nc.vector.tensor_copy(
                    y_sb[:, bb, 2 * s * P + 1 : 2 * (s + 1) * P : 2], pt[:]
                )
                pt2 = tr_ps.tile([P, P], F32R, tag="tr")
                nc.tensor.transpose(pt2[:], x1T[:, s, bsl], ident[:])
                nc.vector.tensor_copy(
                    y_sb[:, bb, 2 * s * P : 2 * (s + 1) * P : 2], pt2[:]
                )
        for bb in range(BC // P):
            nc.sync.dma_start(y_d[bb * P : (bb + 1) * P, :], y_sb[:, bb])

    nc.compile()
    return nc


_NC_CACHE = None


def _get_nc():
    global _NC_CACHE
    if _NC_CACHE is None:
        _NC_CACHE = _build_nc()
    return _NC_CACHE


def kernel(**inputs):
    x = np.ascontiguousarray(np.asarray(inputs["x"], dtype=np.float32))
    ldj = np.ascontiguousarray(np.asarray(inputs["log_det_J"], dtype=np.float32))
    weights = {}
    for i in range(6):
        weights[f"w{i}"] = np.ascontiguousarray(
            np.asarray(inputs[f"w{i}"], dtype=np.float32)
        )
        weights[f"b{i}"] = np.ascontiguousarray(
            np.asarray(inputs[f"b{i}"], dtype=np.float32)
        )

    nc = _get_nc()
    in_maps = []
    for c in range(NCORES):
        m = {"x": x[c * BC : (c + 1) * BC], "log_det_J": ldj[c * BC : (c + 1) * BC]}
        m.update(weights)
        in_maps.append(m)
    res = None
    last_err = None
    for _attempt in range(3):  # NRT_EXEC_UNIT_UNRECOVERABLE is a rare transient
        try:
            res = run_bass_kernel_spmd(nc, in_maps, list(range(NCORES)))
            break
        except Exception as e:  # noqa: BLE001
            last_err = e
            if "UNRECOVERABLE" not in str(e) and "UNAVAILABLE" not in str(e):
                raise
    if res is None:
        raise last_err

    y = np.concatenate([res.results[c]["y"] for c in range(NCORES)], axis=0)
    ld = np.concatenate([res.results[c]["log_det"] for c in range(NCORES)], axis=0)
    return (y, ld)


# revision 22
# speedup vs baseline: 1.2100x; 1.2100x over previous
"""AffineCoupling (NICE) forward on 8 Trainium2 NeuronCores.

Data-parallel over the batch: each core gets a 1024-row slice of x and
runs the full 6-layer MLP (512->2048->2048x4->1024) with replicated
weights, then the coupling epilogue (tanh / exp / scale+shift /
log-det) on chip.

Layout: activations are kept feature-major (hT[feat, batch]) so the
weight matrices, stored [in, out], are directly the stationary matmul
operand and the chain needs no per-layer transposes.  Matmuls run in
fp32r (FP22-precision fp32, full PE rate at N=512).  The even/odd
feature split of the coupling is folded into the M-tiling of the last
layer, so tanh/exp/shift operate on contiguous tiles.
"""

import sys

for _p in ("/opt/trn_rl_repo",):
    if _p not in sys.path:
        sys.path.insert(0, _p)

from contextlib import ExitStack

import numpy as np

import concourse.bass as bass
import concourse.tile as tile
from concourse import bacc, mybir
from concourse.bass_utils import run_bass_kernel_spmd
from concourse.masks import make_identity

P = 128
NCORES = 8
B = 8192  # full batch
W = 1024  # feature width
BC = B // NCORES  # batch rows per core (1024)
NT = 512  # matmul moving-dim tile (max for fp32, = 1 PSUM bank)
MID = 2048
F32 = mybir.dt.float32
F32R = mybir.dt.float32r
AF = mybir.ActivationFunctionType

# (in_dim, out_dim) per layer
L_DIMS = [(W // 2, MID)] + [(MID, MID)] * 4 + [(MID, W)]


def _build_nc():
    nc = bacc.Bacc(trn_type="TRN2", target_bir_lowering=False, debug=False)

    x_d = nc.dram_tensor("x", [BC, W], F32, kind="ExternalInput").ap()
    ldj_d = nc.dram_tensor("log_det_J", [BC], F32, kind="ExternalInput").ap()
    w_d = []
    b_d = []
    for i, (di, do) in enumerate(L_DIMS):
        w_d.append(nc.dram_tensor(f"w{i}", [di, do], F32, kind="ExternalInput").ap())
        b_d.append(nc.dram_tensor(f"b{i}", [do], F32, kind="ExternalInput").ap())
    y_d = nc.dram_tensor("y", [BC, W], F32, kind="ExternalOutput").ap()
    ld_d = nc.dram_tensor("log_det", [BC], F32, kind="ExternalOutput").ap()

    with tile.TileContext(nc) as tc, ExitStack() as ctx:
        const = ctx.enter_context(tc.tile_pool(name="const", bufs=1))
        xt_pool = ctx.enter_context(tc.tile_pool(name="xt", bufs=1))
        xin_pool = ctx.enter_context(tc.tile_pool(name="xin", bufs=3))
        h_pool = ctx.enter_context(tc.tile_pool(name="h", bufs=2))
        w_pool = ctx.enter_context(tc.tile_pool(name="w", bufs=4))
        mm_ps = ctx.enter_context(tc.tile_pool(name="mmps", bufs=4, space="PSUM"))
        tr_ps = ctx.enter_context(tc.tile_pool(name="trps", bufs=2, space="PSUM"))
        ld_ps = ctx.enter_context(tc.tile_pool(name="ldps", bufs=2, space="PSUM"))

        ident_f = const.tile([P, P], F32, tag="identf")
        make_identity(nc, ident_f)
        ident = const.tile([P, P], F32R, tag="ident")
        nc.vector.tensor_copy(ident[:], ident_f[:])
        ones_f = const.tile([P, 1], F32, tag="onesf")
        nc.gpsimd.memset(ones_f[:], 1.0)
        ones = const.tile([P, 1], F32R, tag="ones")
        nc.vector.tensor_copy(ones[:], ones_f[:])

        # Biases, feature-major: column m holds features m*128..m*128+127.
        bias_sb = []
        for l, (_, do) in enumerate(L_DIMS[:5]):
            bt = const.tile([P, do // P], F32, tag=f"bias{l}")
            nc.gpsimd.dma_start(bt[:], b_d[l].rearrange("(mo p) -> p mo", p=P))
            bias_sb.append(bt)
        b5r = b_d[5].rearrange("(mo p two) -> p mo two", p=P, two=2)
        b5e = const.tile([P, 4], F32, tag="b5e")
        nc.gpsimd.dma_start(b5e[:], b5r[:, :, 0])
        b5o = const.tile([P, 4], F32, tag="b5o")
        nc.gpsimd.dma_start(b5o[:], b5r[:, :, 1])
        ld_sb = const.tile([1, BC], F32, tag="ldout")
        nc.gpsimd.dma_start(ld_sb[:], ldj_d.unsqueeze(0))

        # Weight loads: half-pairs [P, <=8, 256] so the first 8 k-chunks are
        # usable while the rest stream; 4 bufs = two m-pairs in flight.
        w_rearr = [w.rearrange("(ko p) m -> p ko m", p=P) for w in w_d]

        def load_pair(l_idx, mp):
            ko = L_DIMS[l_idx][0] // P
            wr = w_rearr[l_idx]
            halves = []
            for h0 in range(0, ko, 8):
                hk = min(8, ko - h0)
                wt = w_pool.tile(
                    [P, 8, 256], F32R, tag="w", name=f"w{l_idx}_{mp}_{h0}"
                )
                nc.sync.dma_start(
                    wt[:, :hk, :],
                    wr[:, h0 : h0 + hk, mp * 256 : (mp + 1) * 256].bitcast(F32R),
                )
                halves.append(wt)
            return halves

        def pair_slice(halves, k, csl):
            return halves[k // 8][:, k % 8, csl]

        def load_l0_quad(mq):
            # L0 (ko=4) packs four m-tiles (512 cols) into one w slot
            wt = w_pool.tile([P, 4, 512], F32R, tag="w", name=f"w0q{mq}")
            nc.sync.dma_start(
                wt[:],
                w_rearr[0][:, 0:4, mq * 512 : (mq + 1) * 512].bitcast(F32R),
            )
            return wt

        l0_quads = {0: load_l0_quad(0)}

        # ---- Stage A: split-transpose x into x1T (even cols) / x2T (odd) ----
        x1T = xt_pool.tile([P, 4, BC], F32R)  # [feat, chunk, batch]
        x2T = xt_pool.tile([P, 4, BC], F32)
        for bb in range(BC // P):
            for hf in range(2):
                xc = xin_pool.tile([P, W // 2], F32R, tag="xblk")
                nc.sync.dma_start(
                    xc[:],
                    x_d[
                        bb * P : (bb + 1) * P, hf * 512 : (hf + 1) * 512
                    ].bitcast(F32R),
                )
                for sh in range(2):  # 256-col spans within this half
                    s = hf * 2 + sh
                    pe = tr_ps.tile([P, P], F32R, tag="tr")
                    nc.tensor.transpose(
                        pe[:], xc[:, sh * 256 : (sh + 1) * 256 : 2], ident[:]
                    )
                    nc.vector.tensor_copy(x1T[:, s, bb * P : (bb + 1) * P], pe[:])
                    po = tr_ps.tile([P, P], F32R, tag="tr")
                    nc.tensor.transpose(
                        po[:], xc[:, sh * 256 + 1 : (sh + 1) * 256 : 2], ident[:]
                    )
                    nc.vector.tensor_copy(x2T[:, s, bb * P : (bb + 1) * P], po[:])
            # trickle the rest of L0's weight loads between x blocks so they
            # don't all queue behind the x DMAs on the Sync engine
            if bb in (1, 3, 5):
                mq = (bb + 1) // 2
                l0_quads[mq] = load_l0_quad(mq)

        cur = x1T
        h5 = None
        for l in range(6):
            di, do = L_DIMS[l]
            ko = di // P
            if l == 0:
                # All weights preloaded as 4 quads; n-outer so the n=0 groups
                # run while the second half of x (bb4-7) is still streaming.
                nxt = h_pool.tile([P, 16, BC], F32R, tag="h")
                for n in range(BC // NT):
                    for mq in range(4):
                        wt = l0_quads[mq]
                        for j in range(4):
                            m = mq * 4 + j
                            acc = mm_ps.tile([P, NT], F32, tag="acc")
                            for k in range(ko):
                                nc.tensor.matmul(
                                    acc[:],
                                    wt[:, k, j * P : (j + 1) * P],
                                    cur[:, k, n * NT : (n + 1) * NT],
                                    start=(k == 0),
                                    stop=(k == ko - 1),
                                )
                            nc.scalar.activation(
                                nxt[:, m, n * NT : (n + 1) * NT],
                                acc[:],
                                AF.Relu,
                                bias=bias_sb[0][:, m : m + 1],
                            )
                cur = nxt
            elif l < 5:
                nxt = h_pool.tile([P, 16, BC], F32R, tag="h")
                for mp in range(do // 256):  # paired m-tiles: 1KB DMA runs
                    wt = load_pair(l, mp)
                    for ms in range(2):
                        m = mp * 2 + ms
                        for n in range(BC // NT):
                            acc = mm_ps.tile([P, NT], F32, tag="acc")
                            for k in range(ko):
                                nc.tensor.matmul(
                                    acc[:],
                                    pair_slice(wt, k, slice(ms * P, (ms + 1) * P)),
                                    cur[:, k, n * NT : (n + 1) * NT],
                                    start=(k == 0),
                                    stop=(k == ko - 1),
                                )
                            nc.scalar.activation(
                                nxt[:, m, n * NT : (n + 1) * NT],
                                acc[:],
                                AF.Relu,
                                bias=bias_sb[l][:, m : m + 1],
                            )
                cur = nxt
            else:
                # Last layer: even out-features -> tanh (log-scale),
                # odd out-features -> identity+bias (shift).
                h5 = h_pool.tile([P, 16, BC], F32R, tag="h")
                b1tT = h5[:, 0:4]  # tanh output, even features
                b2tT = h5[:, 4:8]  # shift, odd features
                laccs = [
                    ld_ps.tile([1, NT], F32, tag="ld", name=f"lacc{n}")
                    for n in range(BC // NT)
                ]
                for m in range(4):  # 256-wide contiguous spans of w5 cols
                    wt = load_pair(l, m)
                    for n in range(BC // NT):
                        nsl = slice(n * NT, (n + 1) * NT)
                        acc_e = mm_ps.tile([P, NT], F32, tag="acc")
                        for k in range(ko):
                            nc.tensor.matmul(
                                acc_e[:],
                                pair_slice(wt, k, slice(0, 256, 2)),
                                cur[:, k, nsl],
                                start=(k == 0),
                                stop=(k == ko - 1),
                            )
                        nc.scalar.activation(
                            b1tT[:, m, nsl],
                            acc_e[:],
                            AF.Tanh,
                            bias=b5e[:, m : m + 1],
                        )
                        acc_o = mm_ps.tile([P, NT], F32, tag="acc")
                        for k in range(ko):
                            nc.tensor.matmul(
                                acc_o[:],
                                pair_slice(wt, k, slice(1, 256, 2)),
                                cur[:, k, nsl],
                                start=(k == 0),
                                stop=(k == ko - 1),
                            )
                        # log_det partial: sum_partitions(tanh chunk) via ones
                        # (after acc_o so the tanh eviction overlaps PE work)
                        nc.tensor.matmul(
                            laccs[n][:],
                            ones[:],
                            b1tT[:, m, nsl],
                            start=(m == 0),
                            stop=(m == 3),
                        )
                        nc.scalar.activation(
                            b2tT[:, m, nsl],
                            acc_o[:],
                            AF.Identity,
                            bias=b5o[:, m : m + 1],
                        )

        assert h5 is not None
        b1tT = h5[:, 0:4]
        b2tT = h5[:, 4:8]
        expb = h5[:, 8:12]
        y2T = h5[:, 12:16]

        # ---- log_det = log_det_J + accumulated tanh partials ----
        for n in range(BC // NT):
            nsl = slice(n * NT, (n + 1) * NT)
            nc.vector.tensor_add(ld_sb[:, nsl], laccs[n][:], ld_sb[:, nsl])
        nc.sync.dma_start(ld_d.unsqueeze(0), ld_sb[:])

        # ---- y2 = x2 * exp(b1t) + b2t (feature-major) ----
        for c in range(4):
            nc.scalar.activation(expb[:, c], b1tT[:, c], AF.Exp)
            nc.vector.tensor_mul(y2T[:, c], expb[:, c], x2T[:, c])
            nc.vector.tensor_add(y2T[:, c], y2T[:, c], b2tT[:, c])

        # ---- Stage D: transpose back + interleave into y ----
        y_combo = h_pool.tile([P, 16, BC], F32, tag="h")
        y_sb = y_combo[:, 0:8]  # [P, batch_block, 1024] batch-major
        for bb in range(BC // P):
            bsl = slice(bb * P, (bb + 1) * P)
            for s in range(4):
                pt = tr_ps.tile([P, P], F32R, tag="tr")
                nc.tensor.transpose(pt[:], y2T[:, s, bsl], ident[:])
                nc.vector.tensor_copy(
                    y_sb[:, bb, 2 * s * P + 1 : 2 * (s + 1) * P : 2], pt[:]
                )
                pt2 = tr_ps.tile([P, P], F32R, tag="tr")
                nc.tensor.transpose(pt2[:], x1T[:, s, bsl], ident[:])
                nc.vector.tensor_copy(
                    y_sb[:, bb, 2 * s * P : 2 * (s + 1) * P : 2], pt2[:]
                )
        for bb in range(BC // P):
            nc.sync.dma_start(y_d[bb * P : (bb + 1) * P, :], y_sb[:, bb])

    nc.compile()
    return nc


_NC_CACHE = None


def _get_nc():
    global _NC_CACHE
    if _NC_CACHE is None:
        _NC_CACHE = _build_nc()
    return _NC_CACHE


def kernel(**inputs):
    x = np.ascontiguousarray(np.asarray(inputs["x"], dtype=np.float32))
    ldj = np.ascontiguousarray(np.asarray(inputs["log_det_J"], dtype=np.float32))
    weights = {}
    for i in range(6):
        weights[f"w{i}"] = np.ascontiguousarray(
            np.asarray(inputs[f"w{i}"], dtype=np.float32)
        )
        weights[f"b{i}"] = np.ascontiguousarray(
            np.asarray(inputs[f"b{i}"], dtype=np.float32)
        )

    nc = _get_nc()
    in_maps = []
    for c in range(NCORES):
        m = {"x": x[c * BC : (c + 1) * BC], "log_det_J": ldj[c * BC : (c + 1) * BC]}
        m.update(weights)
        in_maps.append(m)
    res = None
    last_err = None
    for _attempt in range(3):  # NRT_EXEC_UNIT_UNRECOVERABLE is a rare transient
        try:
            res = run_bass_kernel_spmd(nc, in_maps, list(range(NCORES)))
            break
        except Exception as e:  # noqa: BLE001
            last_err = e
            if "UNRECOVERABLE" not in str(e) and "UNAVAILABLE" not in str(e):
                raise
    if res is None:
        raise last_err

    y = np.concatenate([res.results[c]["y"] for c in range(NCORES)], axis=0)
    ld = np.concatenate([res.results[c]["log_det"] for c in range(NCORES)], axis=0)
    return (y, ld)
